# revision 7
# baseline (speedup 1.0000x reference)
"""Trainium2 Bass kernel for nn_Encoder_48524540511031 (2-layer GNN encoder).

Strategy (8 NeuronCores, SPMD):
  - Shard destination nodes: 6250 nodes/core, padded to 6272 (49 blocks x 128).
  - Edges are grouped by destination block on the host. The segment_sum
    becomes, per dst block, a chain of PE matmuls pooled^T += X^T @ S where
    X = h[src] rows (gathered via batched dma_gather, bf16) and S is a
    host-built one-hot [128 edges x 128 dst] matrix with 1/deg folded in.
  - dma_gather indices are int16, so the (padded) 50176-row node table is
    addressed via two streams: lo (pid < 32768) and hi (pid >= 32768,
    gathered from a sliced source).
  - The MLP runs in transposed layout [feat x node]: z1^T = W1^T @ pooled^T,
    so BatchNorm statistics are free-dim reductions; global stats via a tiny
    AllReduce. BN+ReLU is one scalar-engine activation with per-partition
    scale/bias.
  - Layer end: PE transpose back to node-major; AllGather replicates h for
    the next layer's gather. Final graph mean-pool via one-hot P matmuls
    accumulated in PSUM + AllReduce.
"""

import os
import sys
from contextlib import ExitStack

for _p in ("/opt/trn_rl_repo", "/root/.axon_site/_ro/trn_rl_repo"):
    if os.path.isdir(_p) and _p not in sys.path:
        sys.path.append(_p)

import numpy as np
import ml_dtypes

import concourse.bass as bass
import concourse.mybir as mybir
from concourse import bacc
from concourse.tile import TileContext
from concourse.bass_utils import run_bass_kernel_spmd
from concourse.masks import make_identity

BF16 = ml_dtypes.bfloat16

# Problem constants (hardcoded per spec).
N, E, D, H, L, G = 50000, 800000, 128, 128, 2, 64
BN_EPS = 1e-5
NCORES = 8
SPLIT = 32768                 # int16 index ceiling for dma_gather


class _Cfg:
    def __init__(self, n=N, g=G, bk=32):
        self.N, self.G, self.BK = n, g, bk
        self.NLOC = n // NCORES
        self.NBLK = (self.NLOC + 127) // 128
        self.NLOCP = self.NBLK * 128
        self.NPAD = self.NLOCP * NCORES
        self.LAST_COLS = self.NLOC - (self.NBLK - 1) * 128


_CFG = _Cfg()
_CACHE = {}


# ----------------------------------------------------------------------------
# Host-side preprocessing: graph partitioning + one-hot construction.
# ----------------------------------------------------------------------------

def _wrap_idx(idx, ncols):
    """int16 indices -> wrapped [128, ncols] layout (i -> [i%16, i//16]),
    replicated across the 8 groups of 16 partitions."""
    w = np.zeros((16, ncols), dtype=np.int16)
    n = idx.shape[0]
    w[np.arange(n) % 16, np.arange(n) // 16] = idx
    return np.tile(w, (8, 1))


def _build_plan(cfg, x, edge_src, edge_dst, graph_id):
    NLOC, NBLK, NLOCP = cfg.NLOC, cfg.NBLK, cfg.NLOCP
    BK, Gc = cfg.BK, cfg.G

    deg = np.bincount(edge_dst, minlength=cfg.N).astype(np.float64)
    rdeg = (1.0 / np.maximum(deg, 1.0)).astype(np.float32)
    cnt = np.bincount(graph_id, minlength=Gc).astype(np.float64)
    rcnt = (1.0 / np.maximum(cnt, 1.0)).astype(np.float32)[:, None]  # [G,1]

    core = edge_dst // NLOC                     # destination core per edge
    ldst = edge_dst - core * NLOC               # local dst id
    blk = ldst // 128                           # dst block
    dib = ldst % 128                            # dst slot within block
    spid = (edge_src // NLOC) * NLOCP + (edge_src % NLOC)  # padded src id
    is_hi = spid >= SPLIT

    # Per (core, block, half) edge groups, sorted by source id.
    edges_by = [[[None, None] for _ in range(NBLK)] for _ in range(NCORES)]
    order = np.lexsort((spid, blk, core))
    core_s, blk_s = core[order], blk[order]
    hi_s = is_hi[order]
    for c in range(NCORES):
        m_c = core_s == c
        idx_c = order[m_c]
        blk_c = blk_s[m_c]
        hi_c = hi_s[m_c]
        for b in range(NBLK):
            m_b = blk_c == b
            eb = idx_c[m_b]
            hb = hi_c[m_b]
            edges_by[c][b][0] = eb[~hb]
            edges_by[c][b][1] = eb[hb]

    K = [[0] * NBLK, [0] * NBLK]  # chunks per block per half (shared over cores)
    for b in range(NBLK):
        for h in range(2):
            K[h][b] = max(
                (len(edges_by[c][b][h]) + 127) // 128 for c in range(NCORES)
            )
        if K[0][b] + K[1][b] == 0:
            K[0][b] = 1  # keep at least one (zero) chunk per block
    TK = [sum(K[0]), sum(K[1])]
    off = [np.concatenate([[0], np.cumsum(K[0])]).astype(int),
           np.concatenate([[0], np.cumsum(K[1])]).astype(int)]
    NB = [max(1, -(-TK[0] // BK)), max(1, -(-TK[1] // BK))]  # gather batches

    s_imgs, idx_ws, p_imgs = [], [], []
    for c in range(NCORES):
        s_img = [np.zeros((NB[h], 128, BK * 128), dtype=BF16) for h in range(2)]
        idx_flat = [np.zeros(max(TK[h], 1) * 128, dtype=np.int16) for h in range(2)]
        for b in range(NBLK):
            for h in range(2):
                eb = edges_by[c][b][h]
                ne = len(eb)
                if ne == 0:
                    continue
                c0 = off[h][b]  # first chunk of this (block, half)
                pos = np.arange(ne)
                ch = c0 + pos // 128          # chunk index in stream
                slot = pos % 128              # edge slot within chunk
                idx_flat[h][c0 * 128 + pos] = (
                    (spid[eb] - (SPLIT if h else 0)).astype(np.int16)
                )
                val = rdeg[edge_dst[eb]].astype(BF16)
                s_img[h][ch // BK, slot, (ch % BK) * 128 + dib[eb]] = val
        s_imgs.append(s_img)
        idx_ws.append([
            _wrap_idx(idx_flat[h], NB[h] * BK * 8) for h in range(2)
        ])
        # graph-pool one-hot image [128, NBLK*G]
        pim = np.zeros((128, NBLK * Gc), dtype=np.float32)
        gl = graph_id[c * NLOC:(c + 1) * NLOC]
        ln = np.arange(NLOC)
        pim[ln % 128, (ln // 128) * Gc + gl] = np.float32(1.0)
        p_imgs.append(pim)

    x_pad = np.zeros((cfg.NPAD, D), dtype=BF16)
    for c in range(NCORES):
        x_pad[c * NLOCP:c * NLOCP + NLOC] = x[c * NLOC:(c + 1) * NLOC].astype(BF16)

    return dict(
        K=K, TK=TK, off=off, NB=NB,
        s_imgs=s_imgs, idx_ws=idx_ws, p_imgs=p_imgs,
        x_pad=x_pad, rcnt=rcnt,
    )


# ----------------------------------------------------------------------------
# Device program.
# ----------------------------------------------------------------------------

def _build_nc(cfg, plan, null=False):
    NBLK, NLOCP, NPAD, BK, Gc = cfg.NBLK, cfg.NLOCP, cfg.NPAD, cfg.BK, cfg.G
    LAST_COLS = cfg.LAST_COLS
    K, off, NB = plan["K"], plan["off"], plan["NB"]
    f32, bf16, i16 = mybir.dt.float32, mybir.dt.bfloat16, mybir.dt.int16
    RELU = mybir.ActivationFunctionType.Relu
    SQUARE = mybir.ActivationFunctionType.Square
    SQRT = mybir.ActivationFunctionType.Sqrt
    X = mybir.AxisListType.X

    nc = bacc.Bacc("TRN2", target_bir_lowering=False, debug=False,
                   num_devices=NCORES)

    x_ext = nc.dram_tensor("x_pad", [NPAD, D], bf16, kind="ExternalInput")
    s_ext = [nc.dram_tensor(f"s_img{h}", [NB[h], 128, BK * 128], bf16,
                            kind="ExternalInput") for h in range(2)]
    i_ext = [nc.dram_tensor(f"idx_w{h}", [128, NB[h] * BK * 8], i16,
                            kind="ExternalInput") for h in range(2)]
    w1_ext = nc.dram_tensor("w1", [L, D, H], f32, kind="ExternalInput")
    w2_ext = nc.dram_tensor("w2", [L, H, H], f32, kind="ExternalInput")
    pv_ext = nc.dram_tensor("pv", [128, 12], f32, kind="ExternalInput")
    pim_ext = nc.dram_tensor("p_img", [128, NBLK * Gc], f32, kind="ExternalInput")
    rcnt_ext = nc.dram_tensor("rcnt", [Gc, 1], f32, kind="ExternalInput")

    h_out = nc.dram_tensor("h_out", [cfg.NLOC, D], f32, kind="ExternalOutput")
    pool_out = nc.dram_tensor("pool_out", [Gc, D], f32, kind="ExternalOutput")

    if null:
        # Identical I/O signature, trivial body (for timing differentials).
        with ExitStack() as ctx:
            tc = ctx.enter_context(TileContext(nc))
            pool = ctx.enter_context(tc.tile_pool(name="pool", bufs=1))
            t = pool.tile([Gc, 1], f32)
            nc.sync.dma_start(out=t[:], in_=rcnt_ext[:])
            nc.sync.dma_start(out=pool_out[:Gc, 0:1], in_=t[:])
        nc.compile()
        return nc

    ag_in = nc.dram_tensor("ag_in", [NLOCP, D], bf16)
    h1_full = nc.dram_tensor("h1_full", [NPAD, D], bf16, addr_space="Shared")
    st_in = [nc.dram_tensor(f"st_in{i}", [128, 2], f32) for i in range(4)]
    st_out = [nc.dram_tensor(f"st_out{i}", [128, 2], f32, addr_space="Shared")
              for i in range(4)]
    pg_in = nc.dram_tensor("pg_in", [128, Gc], f32)
    pg_out = nc.dram_tensor("pg_out", [128, Gc], f32, addr_space="Shared")

    rg = [list(range(NCORES))]

    with ExitStack() as ctx:
        tc = ctx.enter_context(TileContext(nc))
        const = ctx.enter_context(tc.tile_pool(name="const", bufs=1))
        persist = ctx.enter_context(tc.tile_pool(name="persist", bufs=1))
        xpool = [ctx.enter_context(tc.tile_pool(name=f"x{h}", bufs=3))
                 for h in range(2)]
        spool = [ctx.enter_context(tc.tile_pool(name=f"s{h}", bufs=3))
                 for h in range(2)]
        work = ctx.enter_context(tc.tile_pool(name="work", bufs=3))
        small = ctx.enter_context(tc.tile_pool(name="small", bufs=4))
        psA = ctx.enter_context(tc.tile_pool(name="psA", bufs=2, space="PSUM"))
        psM = ctx.enter_context(tc.tile_pool(name="psM", bufs=2, space="PSUM"))
        psT = ctx.enter_context(tc.tile_pool(name="psT", bufs=2, space="PSUM"))
        psG = ctx.enter_context(tc.tile_pool(name="psG", bufs=1, space="PSUM"))

        # ---- one-time constant loads ----
        identf = const.tile([128, 128], f32, tag="identf")
        make_identity(nc, identf[:])
        pv = const.tile([128, 12], f32, tag="pv")
        nc.sync.dma_start(out=pv[:], in_=pv_ext[:])
        rcnt_t = const.tile([Gc, 1], f32, tag="rcnt")
        nc.sync.dma_start(out=rcnt_t[:], in_=rcnt_ext[:])
        pim = const.tile([128, NBLK * Gc], f32, tag="pim")
        nc.sync.dma_start(out=pim[:], in_=pim_ext[:])
        w_t = []
        for l in range(L):
            w1t = const.tile([128, 128], f32, tag=f"w1_{l}")
            nc.sync.dma_start(out=w1t[:], in_=w1_ext[l, :, :])
            w2t = const.tile([128, 128], f32, tag=f"w2_{l}")
            nc.sync.dma_start(out=w2t[:], in_=w2_ext[l, :, :])
            w_t.append((w1t, w2t))
        idx_t = []
        for h in range(2):
            it = const.tile([128, NB[h] * BK * 8], i16, tag=f"idx{h}")
            nc.sync.dma_start(out=it[:], in_=i_ext[h][:])
            idx_t.append(it)
        epsc = const.tile([128, 1], f32, tag="epsc")
        nc.vector.memset(epsc[:], BN_EPS)

        def bn_ac(stat_g, gcol, bcol):
            """stat_g [128,2] (sum, sumsq) -> (a, c) [128,1] tiles."""
            mu = small.tile([128, 1], f32, tag="mu")
            nc.scalar.mul(mu[:], stat_g[:, 0:1], 1.0 / cfg.N)
            ex2 = small.tile([128, 1], f32, tag="ex2")
            nc.scalar.mul(ex2[:], stat_g[:, 1:2], 1.0 / cfg.N)
            var = small.tile([128, 1], f32, tag="var")
            nc.vector.tensor_tensor(out=var[:], in0=mu[:], in1=mu[:],
                                    op=mybir.AluOpType.mult)
            nc.vector.tensor_tensor(out=var[:], in0=ex2[:], in1=var[:],
                                    op=mybir.AluOpType.subtract)
            std = small.tile([128, 1], f32, tag="std")
            nc.scalar.activation(out=std[:], in_=var[:], func=SQRT,
                                 bias=epsc[:], scale=1.0)
            rstd = small.tile([128, 1], f32, tag="rstd")
            nc.vector.reciprocal(rstd[:], std[:])
            a_t = small.tile([128, 1], f32, tag="a_t")
            nc.vector.tensor_tensor(out=a_t[:], in0=rstd[:], in1=pv[:, gcol:gcol + 1],
                                    op=mybir.AluOpType.mult)
            c_t = small.tile([128, 1], f32, tag="c_t")
            nc.vector.tensor_tensor(out=c_t[:], in0=a_t[:], in1=mu[:],
                                    op=mybir.AluOpType.mult)
            nc.vector.tensor_tensor(out=c_t[:], in0=pv[:, bcol:bcol + 1], in1=c_t[:],
                                    op=mybir.AluOpType.subtract)
            return a_t, c_t

        def stats_allreduce(sum_parts, sq_parts, slot):
            st = small.tile([128, 2], f32, tag="st_pack")
            nc.vector.reduce_sum(out=st[:, 0:1], in_=sum_parts[:], axis=X)
            nc.vector.reduce_sum(out=st[:, 1:2], in_=sq_parts[:], axis=X)
            nc.sync.dma_start(out=st_in[slot][:], in_=st[:])
            nc.gpsimd.collective_compute(
                "AllReduce", mybir.AluOpType.add, replica_groups=rg,
                ins=[st_in[slot][:]], outs=[st_out[slot][:]])
            stg = small.tile([128, 2], f32, tag="st_glob")
            nc.sync.dma_start(out=stg[:], in_=st_out[slot][:])
            return stg

        pg_acc = None
        for l in range(L):
            hsrc = x_ext if l == 0 else h1_full
            z1T = persist.tile([128, NLOCP], f32, tag="z1T")
            z2T = persist.tile([128, NLOCP], f32, tag="z2T")
            sum1 = persist.tile([128, NBLK], f32, tag="sum1")
            sq1 = persist.tile([128, NBLK], f32, tag="sq1")
            sum2 = persist.tile([128, NBLK], f32, tag="sum2")
            sq2 = persist.tile([128, NBLK], f32, tag="sq2")

            batches = [{}, {}]  # (half) -> {ib: (x_tile, s_tile)}

            def get_batch(h, ib, hsrc=hsrc, batches=batches):
                if ib not in batches[h]:
                    xt = xpool[h].tile([128, BK, 128], bf16, tag="xt")
                    src = hsrc[:, :] if h == 0 else hsrc[SPLIT:, :]
                    nc.gpsimd.dma_gather(
                        out_ap=xt[:], in_ap=src,
                        idxs_ap=idx_t[h][:, ib * BK * 8:(ib + 1) * BK * 8],
                        num_idxs=BK * 128, num_idxs_reg=BK * 128,
                        elem_size=128, single_packet=(BK <= 8))
                    stl = spool[h].tile([128, BK * 128], bf16, tag="st")
                    nc.sync.dma_start(out=stl[:], in_=s_ext[h][ib, :, :])
                    batches[h][ib] = (xt, stl)
                return batches[h][ib]

            # ---- pass A: SpMM + Linear1 + bias, stats of z1 ----
            for b in range(NBLK):
                bs = b * 128
                cols = LAST_COLS if b == NBLK - 1 else 128
                chunks = [(0, c) for c in range(off[0][b], off[0][b] + K[0][b])]
                chunks += [(1, c) for c in range(off[1][b], off[1][b] + K[1][b])]
                pA = psA.tile([128, 128], f32, space="PSUM", tag="pA")
                for j, (h, c) in enumerate(chunks):
                    ib, slot = divmod(c, BK)
                    xt, stl = get_batch(h, ib)
                    nc.tensor.matmul(
                        pA[:], lhsT=xt[:, slot, :],
                        rhs=stl[:, slot * 128:(slot + 1) * 128],
                        start=(j == 0), stop=(j == len(chunks) - 1))
                pooled = work.tile([128, 128], f32, tag="pooled")
                nc.vector.tensor_copy(out=pooled[:], in_=pA[:])
                pM = psM.tile([128, 128], f32, space="PSUM", tag="pM")
                nc.tensor.matmul(pM[:], lhsT=w_t[l][0][:], rhs=pooled[:],
                                 start=True, stop=True)
                nc.scalar.add(z1T[:, bs:bs + 128], pM[:], pv[:, l * 6:l * 6 + 1])
                nc.vector.reduce_sum(out=sum1[:, b:b + 1],
                                     in_=z1T[:, bs:bs + cols], axis=X)
                sq_s = work.tile([128, 128], f32, tag="sq_s")
                nc.scalar.activation(out=sq_s[:, :cols], in_=z1T[:, bs:bs + cols],
                                     func=SQUARE, accum_out=sq1[:, b:b + 1])

            stg1 = stats_allreduce(sum1, sq1, 2 * l)
            a1, c1 = bn_ac(stg1, l * 6 + 1, l * 6 + 2)

            # ---- pass B: BN+ReLU, Linear2 + bias, stats of z2 ----
            for b in range(NBLK):
                bs = b * 128
                cols = LAST_COLS if b == NBLK - 1 else 128
                t2 = work.tile([128, 128], f32, tag="t2")
                nc.scalar.activation(out=t2[:], in_=z1T[:, bs:bs + 128],
                                     func=RELU, bias=c1[:], scale=a1[:])
                pM = psM.tile([128, 128], f32, space="PSUM", tag="pM")
                nc.tensor.matmul(pM[:], lhsT=w_t[l][1][:], rhs=t2[:],
                                 start=True, stop=True)
                nc.scalar.add(z2T[:, bs:bs + 128], pM[:], pv[:, l * 6 + 3:l * 6 + 4])
                nc.vector.reduce_sum(out=sum2[:, b:b + 1],
                                     in_=z2T[:, bs:bs + cols], axis=X)
                sq_s = work.tile([128, 128], f32, tag="sq_s")
                nc.scalar.activation(out=sq_s[:, :cols], in_=z2T[:, bs:bs + cols],
                                     func=SQUARE, accum_out=sq2[:, b:b + 1])

            stg2 = stats_allreduce(sum2, sq2, 2 * l + 1)
            a2, c2 = bn_ac(stg2, l * 6 + 4, l * 6 + 5)

            # ---- pass C: BN+ReLU -> h, transpose to node-major ----
            if l == L - 1:
                pg_acc = psG.tile([128, Gc], f32, space="PSUM", tag="pg")
            for b in range(NBLK):
                bs = b * 128
                cols = LAST_COLS if b == NBLK - 1 else 128
                hTf = work.tile([128, 128], f32, tag="hTf")
                nc.scalar.activation(out=hTf[:], in_=z2T[:, bs:bs + 128],
                                     func=RELU, bias=c2[:], scale=a2[:])
                pT = psT.tile([128, 128], f32, space="PSUM", tag="pT")
                nc.tensor.transpose(out=pT[:], in_=hTf[:], identity=identf[:])
                if l == 0:
                    ndt = work.tile([128, 128], bf16, tag="ndt")
                    nc.vector.tensor_copy(out=ndt[:], in_=pT[:])
                    nc.sync.dma_start(out=ag_in[bs:bs + 128, :], in_=ndt[:])
                else:
                    hof = work.tile([128, 128], f32, tag="hof")
                    nc.vector.tensor_copy(out=hof[:], in_=pT[:])
                    nc.tensor.matmul(pg_acc[:], lhsT=hof[:],
                                     rhs=pim[:, b * Gc:(b + 1) * Gc],
                                     start=(b == 0), stop=(b == NBLK - 1))
                    nc.sync.dma_start(out=h_out[bs:bs + cols, :],
                                      in_=hof[:cols, :])

            if l == 0:
                nc.gpsimd.collective_compute(
                    "AllGather", mybir.AluOpType.bypass, replica_groups=rg,
                    ins=[ag_in[:]], outs=[h1_full[:]])

        # ---- graph pooling epilogue ----
        pgs = small.tile([128, Gc], f32, tag="pgs")
        nc.vector.tensor_copy(out=pgs[:], in_=pg_acc[:])
        nc.sync.dma_start(out=pg_in[:], in_=pgs[:])
        nc.gpsimd.collective_compute(
            "AllReduce", mybir.AluOpType.add, replica_groups=rg,
            ins=[pg_in[:]], outs=[pg_out[:]])
        pgt = small.tile([128, Gc], f32, tag="pgt")
        nc.sync.dma_start(out=pgt[:], in_=pg_out[:])
        pTf = psG.tile([Gc, 128], f32, space="PSUM", tag="pTf")
        nc.tensor.transpose(out=pTf[:], in_=pgt[:], identity=identf[:])
        psc = small.tile([Gc, 128], f32, tag="psc")
        nc.vector.tensor_scalar_mul(psc[:], pTf[:], rcnt_t[:])
        nc.sync.dma_start(out=pool_out[:], in_=psc[:])

    nc.compile()
    return nc


# ----------------------------------------------------------------------------
# Entry point.
# ----------------------------------------------------------------------------

def _prepare(x, edge_src, edge_dst, graph_id, W1, b1, g1, be1, W2, b2, gbn, bbn,
             null=False, cfg=None):
    cfg = cfg or _CFG
    plan = _build_plan(cfg,
                       np.asarray(x, np.float32),
                       np.asarray(edge_src, np.int64),
                       np.asarray(edge_dst, np.int64),
                       np.asarray(graph_id, np.int64))
    key = ("null" if null else "main", cfg.N, cfg.G, cfg.BK,
           tuple(plan["K"][0]), tuple(plan["K"][1]))
    if key not in _CACHE:
        _CACHE[key] = _build_nc(cfg, plan, null=null)
    nc = _CACHE[key]

    pv = np.zeros((128, 12), dtype=np.float32)
    for l in range(L):
        for j, arr in enumerate((b1, g1, be1, b2, gbn, bbn)):
            pv[:, l * 6 + j] = np.asarray(arr, np.float32)[l]

    in_maps = []
    for c in range(NCORES):
        in_maps.append({
            "x_pad": plan["x_pad"],
            "s_img0": plan["s_imgs"][c][0],
            "s_img1": plan["s_imgs"][c][1],
            "idx_w0": plan["idx_ws"][c][0],
            "idx_w1": plan["idx_ws"][c][1],
            "w1": np.asarray(W1, np.float32),
            "w2": np.asarray(W2, np.float32),
            "pv": pv,
            "p_img": plan["p_imgs"][c],
            "rcnt": plan["rcnt"],
        })
    return nc, in_maps


def _run_with_retry(nc, in_maps, tries=3):
    for attempt in range(tries):
        try:
            return run_bass_kernel_spmd(nc, in_maps,
                                        core_ids=list(range(NCORES)))
        except Exception:
            if attempt == tries - 1:
                raise


def kernel(x, edge_src, edge_dst, graph_id, W1, b1, g1, be1, W2, b2, gbn, bbn):
    nc, in_maps = _prepare(x, edge_src, edge_dst, graph_id,
                           W1, b1, g1, be1, W2, b2, gbn, bbn)
    res = _run_with_retry(nc, in_maps)
    h_nodes = np.concatenate(
        [res.results[c]["h_out"] for c in range(NCORES)], axis=0
    ).astype(np.float32)
    pooled_h = res.results[0]["pool_out"].astype(np.float32)
    return pooled_h, h_nodes
